# revision 1
# baseline (speedup 1.0000x reference)
"""Trainium2 Bass kernel for nn_Attention_54391465836966.

Math (per batch b):
  ctok = content_feat[b].reshape(S,C) + pos            # [1024, 512]
  comp_tok[n] = components[n,b].reshape(S,C) + pos
  q = ctok @ Wq ; k[n],v[n] = comp_tok[n] @ Wkv (split)
  per head h, comp n: P = exp(scale * q_h k_h^T); o_nh = (P @ v_nh) / rowsum(P)
  result = sum_n o_n ; s = (result + ctok) @ Wproj + bproj
  out = Wconv[:, :512] @ s2d + Wconv[:, 512:] @ cf2d + bconv
    where s2d = s buffer reinterpreted [512, 1024], cf2d = content_feat[b] as [512, 1024]

Sharding: 8 cores <- (b, n) pairs; b = core//4, n = core%4.  Everything after
`result` is affine in the component partial, so each core applies the linear
tail to its own o_n (the constant terms -- ctok path, biases, cf2d conv -- are
gated to the n==0 core via zeroed per-core inputs) and the host sums the four
partial outputs per batch.  No collectives.

All matmuls run as float32r (~1e-4 relerr, full PE rate).  Scores are computed
transposed (S^T[ki,q]) so softmax-sum lands on the matmul contraction via an
augmented ones-column in V; normalization uses exp(-ln Z) on the ACT engine
(both fns in one table set).
"""
import sys

sys.path.insert(0, "/opt/trn_rl_repo")

import numpy as np

N_CORES = 8
B, C, H, W = 2, 512, 32, 32
S = H * W  # 1024
NH, HD = 8, 64
SCALE = HD ** -0.5

_CACHE = {}


def _build():
    if "nc" in _CACHE:
        return _CACHE["nc"]
    from contextlib import ExitStack

    import concourse.bacc as bacc
    import concourse.mybir as mybir
    import concourse.tile as tile
    from concourse.masks import make_identity

    f32 = mybir.dt.float32
    f32r = mybir.dt.float32r
    EXP = mybir.ActivationFunctionType.Exp

    nc = bacc.Bacc("TRN2", target_bir_lowering=False, debug=False,
                   num_devices=N_CORES)

    # weights / biases declared float32r so they can feed fp32r matmuls
    # straight from DMA (same bits as f32 host-side)
    din = lambda n, s, dt: nc.dram_tensor(n, s, dt, kind="ExternalInput").ap()
    cf = din("cf", [C, S], f32)        # content_feat[b], c-major
    comp = din("comp", [C, S], f32)    # components[n,b], c-major
    pos = din("pos", [S, C], f32)
    wq = din("wq", [C, C], f32r)
    wkv = din("wkv", [C, 2 * C], f32r)
    wproj = din("wproj", [C, C], f32r)
    wconv = din("wconv", [C, 2 * C], f32r)  # cols 512: zeroed for n>0 cores
    bproj = din("bproj", [1, C], f32r)      # zeroed for n>0 cores
    bconv = din("bconv", [1, C], f32r)      # zeroed for n>0 cores
    gate = din("gate", [128, 1], f32)       # 1.0 on n==0 cores else 0.0
    out_p = nc.dram_tensor("out_p", [C, S], f32, kind="ExternalOutput").ap()

    cf_tok = cf.rearrange("a (b c) -> (a b) c", b=2)      # [1024, 512] token view
    comp_tok = comp.rearrange("a (b c) -> (a b) c", b=2)  # [1024, 512]
    cf2d = cf.bitcast(f32r)                                # [512, 1024] c-major

    with tile.TileContext(nc) as tc, ExitStack() as ctx:
        main = ctx.enter_context(tc.tile_pool(name="main", bufs=1))
        trans = ctx.enter_context(tc.tile_pool(name="trans", bufs=2))
        dramp = ctx.enter_context(tc.tile_pool(name="dramp", bufs=1, space="DRAM"))

        # ---- constants ----
        ident = main.tile([128, 128], f32r, tag="ident", name="ident_v17")
        ident32 = trans.tile([128, 128], f32, tag="cn", bufs=4)
        make_identity(nc, ident32[:])
        nc.vector.tensor_copy(ident[:], ident32[:])
        ones = main.tile([128, 512], f32r, tag="ones")
        ones32 = trans.tile([128, 512], f32, tag="cnr", bufs=3)
        nc.gpsimd.memset(ones32[:], 1.0)
        nc.vector.tensor_copy(ones[:], ones32[:])
        g_sb = main.tile([128, 1], f32, tag="g")
        bproj_r = main.tile([1, C], f32r, tag="bpr")
        bconv_r = main.tile([1, C], f32r, tag="bcr")

        # one PSUM pool spans setup + attention so the scheduler can overlap
        # them: mm(1 bank x2) + sc(2 banks x2) + o(2 banks x1) = 8 banks
        with tc.tile_pool(name="psAB", bufs=2, space="PSUM") as ps:
            # ---- token transposes ----
            ctokT = [main.tile([128, S], f32r, tag=f"ctokT{j}", name=f"ctokT{j}")
                     for j in range(4)]
            compT = [main.tile([128, S], f32r, tag=f"cr{j}", name=f"compT{j}",
                               bufs=2) for j in range(4)]
            for t in range(8):
                pos_t = trans.tile([128, C], f32, tag="pos", bufs=3)
                nc.sync.dma_start(pos_t[:], pos[128 * t:128 * (t + 1), :])
                for src, dstT, nm in ((cf_tok, ctokT, "cna"), (comp_tok, compT, "cnb")):
                    nat = trans.tile([128, C], f32, tag="cn", name=nm, bufs=4)
                    nc.sync.dma_start(nat[:], src[128 * t:128 * (t + 1), :])
                    natr = trans.tile([128, C], f32r, tag="cnr", name=nm + "r", bufs=3)
                    # split the pos-adds across DVE and the idle Pool engine
                    if nm == "cna":
                        nc.vector.tensor_add(natr[:], nat[:], pos_t[:])
                    else:
                        nc.gpsimd.tensor_add(natr[:], nat[:], pos_t[:])
                    for j in range(4):
                        tp = ps.tile([128, 128], f32r, tag="mm")
                        nc.tensor.transpose(tp[:], natr[:, 128 * j:128 * (j + 1)],
                                            ident[:])
                        if nm == "cna":
                            nc.vector.tensor_copy(
                                dstT[j][:, 128 * t:128 * (t + 1)], tp[:])
                        else:
                            nc.scalar.copy(dstT[j][:, 128 * t:128 * (t + 1)], tp[:])

            # ---- weights ----
            wq_r = [main.tile([128, C], f32r, tag=f"wq{k}", name=f"wq{k}")
                    for k in range(4)]
            wkv_r = [main.tile([128, 2 * C], f32r, tag=f"wkv{k}", name=f"wkv{k}")
                     for k in range(4)]
            for k in range(4):
                nc.sync.dma_start(wkv_r[k][:], wkv[128 * k:128 * (k + 1), :])
            for k in range(4):
                nc.sync.dma_start(wq_r[k][:], wq[128 * k:128 * (k + 1), :])
            wconvT = [main.tile([128, C], f32r, tag=f"wcT{j}", name=f"wcT{j}")
                      for j in range(8)]
            # late-needed consts: emitted after the token stream so they
            # don't delay the first transposes in the DMA queue
            nc.sync.dma_start(g_sb[:], gate[:])
            nc.sync.dma_start(bproj_r[:], bproj[:])
            nc.sync.dma_start(bconv_r[:], bconv[:])
            # odd heads need Wproj rows at base partition 0 (matmul base
            # rule); even heads read slices of the wproj4 tiles.  The odd
            # tiles ride transient-pool tags that die after startup.
            wproj_odd = []
            for p, (tg, bf) in enumerate((("cn", 4), ("cnr", 3), ("pos", 3))):
                w = trans.tile([64, C], f32r, tag=tg, name=f"wpo{p}", bufs=bf)
                nc.sync.dma_start(w[:], wproj[64 * (2 * p + 1):64 * (2 * p + 2), :])
                wproj_odd.append(w)

            # ---- v (first: gates attention start), then kT/qT ----
            v_sb = [main.tile([128, 8 * 65], f32r, tag=f"v{t}", name=f"v{t}")
                    for t in range(8)]
            for t in range(8):
                nc.scalar.copy(
                    v_sb[t][:].rearrange("p (h e) -> p h e", h=8)[:, :, 64:65],
                    ones[:, 0:8].rearrange("p (h o) -> p h o", o=1))
                acc = ps.tile([128, 512], f32, tag="mm")
                for k in range(4):
                    nc.tensor.matmul(acc[:], compT[k][:, 128 * t:128 * (t + 1)],
                                     wkv_r[k][:, C:2 * C],
                                     start=(k == 0), stop=(k == 3))
                nc.scalar.copy(
                    v_sb[t][:].rearrange("p (h e) -> p h e", h=8)[:, :, 0:64],
                    acc[:].rearrange("p (h d) -> p h d", h=8))

            qT = [main.tile([128, S], f32r, tag=f"qT{j}", name=f"qT{j}")
                  for j in range(4)]
            kT = [main.tile([128, S], f32r, tag=f"kT{j}", name=f"kT{j}")
                  for j in range(4)]
            for j in range(4):
                for tck in range(2):
                    for dst, wsrc, act in ((kT, wkv_r, compT), (qT, wq_r, ctokT)):
                        acc = ps.tile([128, 512], f32, tag="mm")
                        for k in range(4):
                            nc.tensor.matmul(acc[:],
                                             wsrc[k][:, 128 * j:128 * (j + 1)],
                                             act[k][:, 512 * tck:512 * (tck + 1)],
                                             start=(k == 0), stop=(k == 3))
                        nc.vector.tensor_copy(
                            dst[j][:, 512 * tck:512 * (tck + 1)], acc[:])

            # Wconv transpose (cheap PE; copies on DVE)
            for i in range(4):
                wcr = trans.tile([128, 2 * C], f32r, tag="wcr", bufs=1)
                nc.sync.dma_start(wcr[:], wconv[128 * i:128 * (i + 1), :])
                for j in range(8):
                    tp = ps.tile([128, 128], f32r, tag="mm")
                    nc.tensor.transpose(tp[:], wcr[:, 128 * j:128 * (j + 1)],
                                        ident[:])
                    nc.vector.tensor_copy(wconvT[j][:, 128 * i:128 * (i + 1)],
                                          tp[:])

            # ---- attention + interleaved per-head normalization ----
            rTu = [main.tile([65, S], f32r, tag=f"cr{h // 2}", name=f"rTu{h}",
                             bufs=2) for h in range(NH)]
            zscr = main.tile([65, S], f32, tag="zscr")
            zinv = main.tile([65, S], f32, tag="zinv")
            zs2 = main.tile([1, S], f32, tag="zs2")
            for h in range(NH):
                jq, row = h // 2, 64 * (h % 2)
                o_ps = ps.tile([65, S], f32, tag="o", bufs=1)
                for kt in range(8):
                    sc = ps.tile([128, S], f32, tag="sc", bufs=2)
                    for qc in range(2):
                        nc.tensor.matmul(
                            sc[:, 512 * qc:512 * (qc + 1)],
                            kT[jq][row:row + 64, 128 * kt:128 * (kt + 1)],
                            qT[jq][row:row + 64, 512 * qc:512 * (qc + 1)],
                            start=True, stop=True)
                    pt = main.tile([128, S], f32r, tag=f"wq{kt % 4}",
                                   name=f"pt{kt}")
                    nc.scalar.activation(pt[:], sc[:], EXP, scale=SCALE)
                    for qc in range(2):
                        nc.tensor.matmul(
                            o_ps[:, 512 * qc:512 * (qc + 1)],
                            v_sb[kt][:, 65 * h:65 * h + 65],
                            pt[:, 512 * qc:512 * (qc + 1)],
                            start=(kt == 0), stop=(kt == 7))
                # custom-DVE recip and partition_broadcast both need base
                # partition 0 on HW: shift the Z row down first (1-input
                # copies may change base partition).  For the last head the
                # recip chain starts straight from PSUM on DVE while the rTu
                # copy runs on the idle ACT engine, shortening the tail gate.
                if h == NH - 1:
                    nc.vector.tensor_copy(zscr[0:1, :], o_ps[64:65, :])
                    nc.scalar.copy(rTu[h][:], o_ps[:])
                else:
                    nc.vector.tensor_copy(rTu[h][:], o_ps[:])
                    nc.vector.tensor_copy(zscr[0:1, :], rTu[h][64:65, :])
                nc.vector.reciprocal_approx_accurate(
                    zinv[0:1, :], zscr[0:1, :], zs2[0:1, :])
                nc.gpsimd.partition_broadcast(zscr[0:64, :], zinv[0:1, :])
                mul_eng = nc.vector if h == NH - 1 else nc.gpsimd
                mul_eng.tensor_mul(rTu[h][0:64, :], rTu[h][0:64, :],
                                   zscr[0:64, :])

            wpo3 = trans.tile([64, C], f32r, tag="wcr", name="wpo3", bufs=1)
            nc.sync.dma_start(wpo3[:], wproj[64 * 7:64 * 8, :])
            wproj_odd.append(wpo3)

            # ---- cf2d partial conv: emitted after attention so it fills the
            # ACT-bound PE gaps; rides the dead wkv tags ----
            cf2d_r = [main.tile([128, S], f32r, tag=f"wkv{j}", name=f"c2r{j}")
                      for j in range(4)]
            for j in range(4):
                nc.sync.dma_start(cf2d_r[j][:], cf2d[128 * j:128 * (j + 1), :])
            outpart = [main.tile([128, S], f32, tag=f"op{oc}", name=f"op{oc}")
                       for oc in range(4)]
            for oc in range(4):
                for pc in range(2):
                    acc = ps.tile([128, 512], f32, tag="mm")
                    nc.tensor.matmul(acc[:], bconv_r[0:1, 128 * oc:128 * (oc + 1)],
                                     ones[0:1, :], start=True, stop=False)
                    for k2 in range(4):
                        nc.tensor.matmul(acc[:],
                                         wconvT[4 + k2][:, 128 * oc:128 * (oc + 1)],
                                         cf2d_r[k2][:, 512 * pc:512 * (pc + 1)],
                                         start=False, stop=(k2 == 3))
                    nc.vector.tensor_copy(outpart[oc][:, 512 * pc:512 * (pc + 1)],
                                          acc[:])

        # gate ctokT in place (only read by the proj matmuls afterwards)
        for j in range(4):
            nc.vector.tensor_scalar_mul(ctokT[j][:], ctokT[j][:], g_sb[:, 0:1])
        # second copy of Wproj in 4x[128,C] layout for the gated-ctok proj
        # terms; rides the wkv tags after cf2d
        wproj4 = [main.tile([128, C], f32r, tag=f"wkv{j}", name=f"wp4_{j}")
                  for j in range(4)]
        for j in range(4):
            nc.sync.dma_start(wproj4[j][:], wproj[128 * j:128 * (j + 1), :])

        # ---- proj + conv tail (pipelined through DRAM in 4 chunks) ----
        st_dram = [dramp.tile([128, C], f32r, name=f"stt{t}") for t in range(8)]
        with tc.tile_pool(name="psC", bufs=2, space="PSUM") as psC:
            for t in range(8):
                acc = psC.tile([128, 512], f32, tag="mm2")
                nc.tensor.matmul(acc[:], ones[0:1, 0:128], bproj_r[:],
                                 start=True, stop=False)
                for j in range(4):
                    nc.tensor.matmul(acc[:],
                                     ctokT[j][:, 128 * t:128 * (t + 1)],
                                     wproj4[j][:], start=False, stop=False)
                for h in range(NH):
                    wp_rhs = (wproj4[h // 2][0:64, :] if h % 2 == 0
                              else wproj_odd[h // 2][:])
                    nc.tensor.matmul(acc[:],
                                     rTu[h][0:64, 128 * t:128 * (t + 1)],
                                     wp_rhs,
                                     start=False, stop=(h == NH - 1))
                st = main.tile([128, C], f32r, tag=f"st{t % 2}", name=f"st{t}")
                nc.scalar.copy(st[:], acc[:])
                nc.sync.dma_start(st_dram[t][:, :], st[:])

            # each half of an s2d chunk depends on only one proj tile's store,
            # so the reload pipelines per-tile instead of per-chunk
            s2d_sb = []
            for j in range(4):
                sj = main.tile([128, S], f32r, tag=f"qT{j}", name=f"s2d{j}")
                for half in range(2):
                    hv = st_dram[2 * j + half][:].rearrange(
                        "(a b) c -> a (b c)", a=64, b=2)
                    nc.sync.dma_start(sj[64 * half:64 * half + 64, :], hv[:, :])
                s2d_sb.append(sj)
            for oc in range(4):
                for pc in range(2):
                    acc = psC.tile([128, 512], f32, tag="cv", bufs=6)
                    for j in range(4):
                        nc.tensor.matmul(acc[:],
                                         wconvT[j][:, 128 * oc:128 * (oc + 1)],
                                         s2d_sb[j][:, 512 * pc:512 * (pc + 1)],
                                         start=(j == 0), stop=(j == 3))
                    nc.vector.tensor_add(
                        outpart[oc][:, 512 * pc:512 * (pc + 1)],
                        outpart[oc][:, 512 * pc:512 * (pc + 1)], acc[:])
                    nc.sync.dma_start(
                        out_p[128 * oc:128 * (oc + 1),
                              512 * pc:512 * (pc + 1)],
                        outpart[oc][:, 512 * pc:512 * (pc + 1)])

    nc.compile()
    _CACHE["nc"] = nc
    return nc


def _shard_inputs(content_feat, components, pos_emb, Wq, Wkv, Wproj, bproj,
                  Wconv, bconv):
    f = np.float32
    pos2 = np.ascontiguousarray(pos_emb.reshape(S, C), dtype=f)
    wq2 = np.ascontiguousarray(Wq, dtype=f)
    wkv2 = np.ascontiguousarray(Wkv, dtype=f)
    wp2 = np.ascontiguousarray(Wproj, dtype=f)
    wc_first = np.ascontiguousarray(Wconv, dtype=f)
    wc_rest = wc_first.copy()
    wc_rest[:, C:] = 0.0
    bp1 = np.ascontiguousarray(bproj.reshape(1, C), dtype=f)
    bc1 = np.ascontiguousarray(bconv.reshape(1, C), dtype=f)
    zeros1 = np.zeros((1, C), dtype=f)
    in_maps = []
    for core in range(N_CORES):
        b, n = core // 4, core % 4
        first = n == 0
        in_maps.append({
            "cf": np.ascontiguousarray(content_feat[b].reshape(C, S), dtype=f),
            "comp": np.ascontiguousarray(components[n, b].reshape(C, S), dtype=f),
            "pos": pos2,
            "wq": wq2,
            "wkv": wkv2,
            "wproj": wp2,
            "wconv": wc_first if first else wc_rest,
            "bproj": bp1 if first else zeros1,
            "bconv": bc1 if first else zeros1,
            "gate": np.full((128, 1), 1.0 if first else 0.0, dtype=f),
        })
    return in_maps


def _run(trace=False, **inputs):
    from concourse.bass_utils import run_bass_kernel_spmd

    nc = _build()
    in_maps = _shard_inputs(**inputs)
    res = run_bass_kernel_spmd(nc, in_maps, list(range(N_CORES)), trace=trace)
    outs = [res.results[i]["out_p"] for i in range(N_CORES)]
    out = np.stack([outs[0] + outs[1] + outs[2] + outs[3],
                    outs[4] + outs[5] + outs[6] + outs[7]], axis=0)
    return out.reshape(B, C, H, W).astype(np.float32), res


def kernel(**inputs):
    out, _ = _run(trace=False, **inputs)
    return out



# revision 6
# speedup vs baseline: 1.1354x; 1.1354x over previous
"""Trainium2 Bass kernel for nn_Attention_54391465836966.

Math (per batch b):
  ctok = content_feat[b].reshape(S,C) + pos            # [1024, 512]
  comp_tok[n] = components[n,b].reshape(S,C) + pos
  q = ctok @ Wq ; k[n],v[n] = comp_tok[n] @ Wkv (split)
  per head h, comp n: P = exp(scale * q_h k_h^T); o_nh = (P @ v_nh) / rowsum(P)
  result = sum_n o_n ; s = (result + ctok) @ Wproj + bproj
  out = Wconv^T[:512] @ s2d + Wconv^T[512:] @ cf2d + bconv
    where s2d = s buffer raw-reshaped [512, 1024], cf2d = content_feat[b] [512, 1024]

Sharding: 8 cores <- (b, n) pairs; b = core//4, n = core%4.  Everything after
`result` is affine in the component partial, so each core applies the linear
tail to its own o_n (constant terms -- ctok path, biases, cf2d conv -- gated
to the n==0 core via zeroed per-core inputs) and the host sums the four
partial outputs per batch.  No collectives.

Layout strategy (v2): everything stays channel-major on chip -- the host
pre-transposes pos_emb and Wconv, so no token transposes are needed at all.
All activations and weights are bf16 (halves the DMA-bound startup); PSUM
accumulation is fp32.  The projection is computed transposed
(stT = Wproj^T @ s_in^T) with head pairs packed into 128-partition tiles so
every matmul contracts over a full 128 partitions.  The raw-reshape s->s2d
repack is done with 32 PE transposes of stride-2 column slices (even/odd
tokens) instead of a DRAM round trip.  Biases are applied as per-partition
tensor_scalar adds on the PSUM->SBUF copies (no bias matmuls).
"""
import sys

sys.path.insert(0, "/opt/trn_rl_repo")

import numpy as np

N_CORES = 8
B, C, H, W = 2, 512, 32, 32
S = H * W  # 1024
NH, HD = 8, 64
SCALE = HD ** -0.5

_CACHE = {}

# fallback switch: if the base-shifted odd-head copy misbehaves on HW, set
# False to use unpaired (K=64) projection matmuls instead.
PAIRED_PROJ = True


def _build():
    if "nc" in _CACHE:
        return _CACHE["nc"]
    from contextlib import ExitStack

    import concourse.bacc as bacc
    import concourse.mybir as mybir
    import concourse.tile as tile
    from concourse.masks import make_identity

    f32 = mybir.dt.float32
    bf16 = mybir.dt.bfloat16
    EXP = mybir.ActivationFunctionType.Exp
    MULT = mybir.AluOpType.mult
    ADD = mybir.AluOpType.add

    nc = bacc.Bacc("TRN2", target_bir_lowering=False, debug=False,
                   num_devices=N_CORES)

    din = lambda n, s, dt: nc.dram_tensor(n, s, dt, kind="ExternalInput").ap()
    cf = din("cf", [C, S], bf16)         # content_feat[b] raw [C,S] (conv only)
    ctokTd = din("ctokT", [C, S], bf16)  # (content_tok + pos).T, host-prepped
    compTd = din("compT", [C, S], bf16)  # (comp_tok + pos).T, host-prepped
    wq = din("wq", [C, C], bf16)
    wkv = din("wkv", [C, 2 * C], bf16)   # cols 0:C -> K, C:2C -> V
    wproj = din("wproj", [C, C], bf16)
    wconvT = din("wconvT", [2 * C, C], bf16)  # Wconv.T; rows C: zeroed n>0
    bprojT = din("bprojT", [C, 1], f32)       # zeroed n>0
    bconvT = din("bconvT", [C, 1], f32)       # zeroed n>0
    gate = din("gate", [128, 1], f32)         # 1.0 on n==0 cores else 0.0
    out_p = nc.dram_tensor("out_p", [C, S], f32, kind="ExternalOutput").ap()

    with tile.TileContext(nc) as tc, ExitStack() as ctx:
        main = ctx.enter_context(tc.tile_pool(name="main", bufs=1))

        # ---- constants ----
        ident32 = main.tile([128, 128], f32, tag="id32")
        make_identity(nc, ident32[:])
        ident = main.tile([128, 128], bf16, tag="ident")
        nc.vector.tensor_copy(ident[:], ident32[:])
        ones_bf = main.tile([128, 8], bf16, tag="ones")
        nc.gpsimd.memset(ones_bf[:], 1.0)
        g_sb = main.tile([128, 1], f32, tag="g")
        bpj_sb = [main.tile([128, 1], f32, tag=f"bpj{i}", name=f"bpj{i}") for i in range(4)]
        bcv_sb = [main.tile([128, 1], f32, tag=f"bcv{i}", name=f"bcv{i}") for i in range(4)]

        # ---- persistent SBUF tiles ----
        wkv_sb = [main.tile([128, 2 * C], bf16, tag=f"wkv{k}", name=f"wkv{k}") for k in range(4)]
        wq_sb = [main.tile([128, C], bf16, tag=f"wq{k}", name=f"wq{k}") for k in range(4)]
        comp_sb = [main.tile([128, S], bf16, tag=f"cm{k}", name=f"cm{k}") for k in range(4)]
        cf_sb = [main.tile([128, S], bf16, tag=f"cf{k}", name=f"cf{k}") for k in range(4)]
        ctokT = [main.tile([128, S], bf16, tag=f"ct{k}", name=f"ct{k}") for k in range(4)]
        kT = [main.tile([128, S], bf16, tag=f"kt{j}", name=f"kt{j}") for j in range(4)]
        qT = [main.tile([128, S], bf16, tag=f"qt{j}", name=f"qt{j}") for j in range(4)]
        v_sb = [main.tile([128, 8 * 65], bf16, tag=f"v{t}", name=f"v{t}") for t in range(8)]
        wcc_sb = [main.tile([128, C], bf16, tag=f"wcc{k}", name=f"wcc{k}") for k in range(4)]
        wcs_sb = [main.tile([128, C], bf16, tag=f"wcs{k}", name=f"wcs{k}") for k in range(4)]
        wp_sb = [main.tile([128, C], bf16, tag=f"wp{k}", name=f"wp{k}") for k in range(4)]
        outpart = [main.tile([128, S], f32, tag=f"op{oc}", name=f"op{oc}") for oc in range(4)]

        # ---- DMA emission order: attention-critical first ----
        for k in range(4):
            nc.sync.dma_start(comp_sb[k][:], compTd[128 * k:128 * (k + 1), :])
            nc.sync.dma_start(wkv_sb[k][:], wkv[128 * k:128 * (k + 1), :])
        for k in range(4):
            nc.sync.dma_start(wq_sb[k][:], wq[128 * k:128 * (k + 1), :])
        for k in range(4):
            nc.sync.dma_start(ctokT[k][:], ctokTd[128 * k:128 * (k + 1), :])
        for k in range(4):
            nc.sync.dma_start(cf_sb[k][:], cf[128 * k:128 * (k + 1), :])
        for k in range(4):  # conv weights for the cf half (rows C:2C of Wconv.T)
            nc.sync.dma_start(wcc_sb[k][:], wconvT[C + 128 * k:C + 128 * (k + 1), :])
        nc.sync.dma_start(g_sb[:], gate[:])
        for i in range(4):
            nc.sync.dma_start(bcv_sb[i][:], bconvT[128 * i:128 * (i + 1), :])
            nc.sync.dma_start(bpj_sb[i][:], bprojT[128 * i:128 * (i + 1), :])
        for k in range(4):  # tail weights last
            nc.sync.dma_start(wp_sb[k][:], wproj[128 * k:128 * (k + 1), :])
        for k in range(4):
            nc.sync.dma_start(wcs_sb[k][:], wconvT[128 * k:128 * (k + 1), :])

        with tc.tile_pool(name="psStart", bufs=1, space="PSUM") as psS:
            # kT[0] first: earliest PE work (needs only comp+pos+wkv)
            def emit_kq(j, dst, wsrc, act, cp_eng):
                for tck in range(2):
                    acc = psS.tile([128, 512], f32, tag="mm", bufs=2)
                    for k in range(4):
                        nc.tensor.matmul(acc[:],
                                         wsrc[k][:, 128 * j:128 * (j + 1)],
                                         act[k][:, 512 * tck:512 * (tck + 1)],
                                         start=(k == 0), stop=(k == 3))
                    cp_eng(dst[j][:, 512 * tck:512 * (tck + 1)], acc[:])

            emit_kq(0, kT, wkv_sb, comp_sb, nc.vector.tensor_copy)

            # v (lhsT for all heads' o matmuls)
            for t in range(8):
                nc.scalar.copy(
                    v_sb[t][:].rearrange("p (h e) -> p h e", h=8)[:, :, 64:65],
                    ones_bf[:, 0:8].rearrange("p (h o) -> p h o", o=1))
                acc = psS.tile([128, 512], f32, tag="mm", bufs=2)
                for k in range(4):
                    nc.tensor.matmul(acc[:], comp_sb[k][:, 128 * t:128 * (t + 1)],
                                     wkv_sb[k][:, C:2 * C],
                                     start=(k == 0), stop=(k == 3))
                nc.scalar.copy(
                    v_sb[t][:].rearrange("p (h e) -> p h e", h=8)[:, :, 0:64],
                    acc[:].rearrange("p (h d) -> p h d", h=8))

            for j in range(1, 4):
                emit_kq(j, kT, wkv_sb, comp_sb, nc.vector.tensor_copy)
            for j in range(4):
                emit_kq(j, qT, wq_sb, ctokT, nc.scalar.copy)

            # conv over the cf half, first 2 output-channel chunks (the other
            # two are emitted after attention to fill the tail norm bubble)
            for oc in range(2):
                for pc in range(2):
                    acc = psS.tile([128, 512], f32, tag="mm", bufs=2)
                    for k2 in range(4):
                        nc.tensor.matmul(acc[:],
                                         wcc_sb[k2][:, 128 * oc:128 * (oc + 1)],
                                         cf_sb[k2][:, 512 * pc:512 * (pc + 1)],
                                         start=(k2 == 0), stop=(k2 == 3))
                    nc.vector.tensor_scalar_add(
                        outpart[oc][:, 512 * pc:512 * (pc + 1)], acc[:],
                        bcv_sb[oc][:, 0:1])

        # ---- attention ----
        # rTu2[j] holds the pair (2j, 2j+1): rows 0:64 even head, 64:128 odd.
        # After normalization, s_in[j] = rTu2[j] + gate*ctokT[j] in place.
        rTu2 = [main.tile([128, S], bf16, tag=f"rt{j}", name=f"rt{j}") for j in range(4)]
        with tc.tile_pool(name="psAttn", bufs=1, space="PSUM") as psA:
            sc_prev = None
            for h in range(NH):
                jq, row = h // 2, 64 * (h % 2)
                o_ps = psA.tile([65, S], f32, tag="o", bufs=2)
                scs = []
                for kt in range(8):
                    if h == 0 or kt > 0:
                        sc = psA.tile([128, S], f32, tag="sc", bufs=2)
                        for qc in range(2):
                            nc.tensor.matmul(
                                sc[:, 512 * qc:512 * (qc + 1)],
                                kT[jq][row:row + 64, 128 * kt:128 * (kt + 1)],
                                qT[jq][row:row + 64, 512 * qc:512 * (qc + 1)],
                                start=True, stop=True)
                    else:
                        sc = sc_prev  # kt=0 scores were emitted in prev head
                    scs.append(sc)
                # software pipeline: emit scores(kt+1) before o(kt) so PE
                # never waits on the ACT exp of tile kt
                pts = []
                for kt in range(8):
                    if h < NH - 1 and kt == 7:
                        # pre-emit next head's kt=0 scores
                        jq2, row2 = (h + 1) // 2, 64 * ((h + 1) % 2)
                        sc_prev = psA.tile([128, S], f32, tag="sc", bufs=2)
                        for qc in range(2):
                            nc.tensor.matmul(
                                sc_prev[:, 512 * qc:512 * (qc + 1)],
                                kT[jq2][row2:row2 + 64, 0:128],
                                qT[jq2][row2:row2 + 64,
                                        512 * qc:512 * (qc + 1)],
                                start=True, stop=True)
                    pt = main.tile([128, S], bf16, tag=f"pt{kt % 4}",
                                   name=f"pt{h}_{kt}")
                    nc.scalar.activation(pt[:], scs[kt][:], EXP, scale=SCALE)
                    pts.append(pt)
                    for qc in range(2):
                        nc.tensor.matmul(
                            o_ps[:, 512 * qc:512 * (qc + 1)],
                            v_sb[kt][:, 65 * h:65 * h + 65],
                            pt[:, 512 * qc:512 * (qc + 1)],
                            start=(kt == 0), stop=(kt == 7))
                # normalization: Z row (64) -> recip -> broadcast -> fused
                # copy*zinv into the pair tile
                zE = main.tile([1, S], f32, tag="z", bufs=2)
                zinv = main.tile([1, S], f32, tag="zi", bufs=2)
                zbc = main.tile([64, S], f32, tag="zb", bufs=2)
                nc.vector.tensor_copy(zE[0:1, :], o_ps[64:65, :])
                nc.vector.reciprocal_approx_fast(zinv[0:1, :], zE[0:1, :])
                nc.gpsimd.partition_broadcast(zbc[0:64, :], zinv[0:1, :])
                if h % 2 == 0:
                    nc.vector.tensor_mul(rTu2[jq][0:64, :], o_ps[0:64, :],
                                         zbc[0:64, :])
                else:
                    oscr = main.tile([64, S], bf16, tag="osc", bufs=2)
                    nc.vector.tensor_mul(oscr[0:64, :], o_ps[0:64, :],
                                         zbc[0:64, :])
                    # 1-input copy with base-partition shift 0 -> 64
                    nc.vector.tensor_copy(rTu2[jq][64:128, :], oscr[0:64, :])
                    # s_in[j] = rTu2[j] + gate * ctokT[j]
                    nc.vector.scalar_tensor_tensor(
                        rTu2[jq][:], ctokT[jq][:], g_sb[:, 0:1], rTu2[jq][:],
                        MULT, ADD)

        # ---- tail: transposed proj, s2d transposes, conv over s half ----
        stT = [main.tile([128, S], bf16, tag=f"cm{cc}", name=f"stT{cc}")
               for cc in range(4)]
        s2d = [main.tile([128, S], bf16, tag=f"kt{jj}", name=f"s2d{jj}")
               for jj in range(4)]
        with tc.tile_pool(name="psTail", bufs=1, space="PSUM") as psT:
            # remaining cf-half conv (fills PE while the last pair normalizes)
            for oc in range(2, 4):
                for pc in range(2):
                    acc = psT.tile([128, 512], f32, tag="cva", bufs=2)
                    for k2 in range(4):
                        nc.tensor.matmul(acc[:],
                                         wcc_sb[k2][:, 128 * oc:128 * (oc + 1)],
                                         cf_sb[k2][:, 512 * pc:512 * (pc + 1)],
                                         start=(k2 == 0), stop=(k2 == 3))
                    nc.vector.tensor_scalar_add(
                        outpart[oc][:, 512 * pc:512 * (pc + 1)], acc[:],
                        bcv_sb[oc][:, 0:1])

            # stT[cc] = Wproj^T @ s_in^T  (+ bproj per partition)
            for cc in range(4):
                for half in range(2):
                    acc = psT.tile([128, 512], f32, tag="st", bufs=2)
                    for j in range(4):
                        nc.tensor.matmul(
                            acc[:],
                            wp_sb[j][:, 128 * cc:128 * (cc + 1)],
                            rTu2[j][:, 512 * half:512 * (half + 1)],
                            start=(j == 0), stop=(j == 3))
                    nc.vector.tensor_scalar_add(
                        stT[cc][:, 512 * half:512 * (half + 1)], acc[:],
                        bpj_sb[cc][:, 0:1])

            # s2d repack: s2d[i, c + 512*par] = stT[c, 2i + par]
            cp_engs = [nc.scalar.copy, nc.vector.tensor_copy]
            for cc in range(4):
                ev = stT[cc][:].rearrange("p (t two) -> p two t", two=2)
                for jj in range(4):
                    for par in range(2):
                        tp = psT.tile([128, 128], bf16, tag="tp", bufs=2)
                        nc.tensor.transpose(
                            tp[:], ev[:, par, 128 * jj:128 * (jj + 1)],
                            ident[:])
                        cp_engs[(cc * 8 + jj * 2 + par) % 2](
                            s2d[jj][:, 512 * par + 128 * cc:
                                    512 * par + 128 * (cc + 1)], tp[:])

            # conv over the s half; accumulate into outpart and store
            for oc in range(4):
                for pc in range(2):
                    acc = psT.tile([128, 512], f32, tag="cva", bufs=2)
                    for jj in range(4):
                        nc.tensor.matmul(acc[:],
                                         wcs_sb[jj][:, 128 * oc:128 * (oc + 1)],
                                         s2d[jj][:, 512 * pc:512 * (pc + 1)],
                                         start=(jj == 0), stop=(jj == 3))
                    nc.vector.tensor_add(
                        outpart[oc][:, 512 * pc:512 * (pc + 1)],
                        outpart[oc][:, 512 * pc:512 * (pc + 1)], acc[:])
                    nc.sync.dma_start(
                        out_p[128 * oc:128 * (oc + 1),
                              512 * pc:512 * (pc + 1)],
                        outpart[oc][:, 512 * pc:512 * (pc + 1)])

    nc.compile()
    _CACHE["nc"] = nc
    return nc


def _shard_inputs(content_feat, components, pos_emb, Wq, Wkv, Wproj, bproj,
                  Wconv, bconv):
    import ml_dtypes

    bf = ml_dtypes.bfloat16
    f = np.float32
    pos2 = np.asarray(pos_emb, dtype=f).reshape(S, C)
    wq2 = np.asarray(Wq, dtype=f).astype(bf)
    wkv2 = np.asarray(Wkv, dtype=f).astype(bf)
    wp2 = np.asarray(Wproj, dtype=f).astype(bf)
    wcT = np.ascontiguousarray(np.asarray(Wconv, dtype=f).T).astype(bf)
    wcT_rest = wcT.copy()
    wcT_rest[C:, :] = 0
    bpj = np.ascontiguousarray(np.asarray(bproj, dtype=f).reshape(C, 1))
    bcv = np.ascontiguousarray(np.asarray(bconv, dtype=f).reshape(C, 1))
    zeros = np.zeros((C, 1), dtype=f)
    in_maps = []
    for core in range(N_CORES):
        b, n = core // 4, core % 4
        first = n == 0
        in_maps.append({
            "cf": np.ascontiguousarray(
                np.asarray(content_feat[b], dtype=f).reshape(C, S)).astype(bf),
            "ctokT": np.ascontiguousarray(
                (np.asarray(content_feat[b], dtype=f).reshape(S, C)
                 + pos2).T).astype(bf),
            "compT": np.ascontiguousarray(
                (np.asarray(components[n, b], dtype=f).reshape(S, C)
                 + pos2).T).astype(bf),
            "wq": wq2,
            "wkv": wkv2,
            "wproj": wp2,
            "wconvT": wcT if first else wcT_rest,
            "bprojT": bpj if first else zeros,
            "bconvT": bcv if first else zeros,
            "gate": np.full((128, 1), 1.0 if first else 0.0, dtype=f),
        })
    return in_maps


def _run(trace=False, **inputs):
    from concourse.bass_utils import run_bass_kernel_spmd

    nc = _build()
    in_maps = _shard_inputs(**inputs)
    res = run_bass_kernel_spmd(nc, in_maps, list(range(N_CORES)), trace=trace)
    outs = [np.asarray(res.results[i]["out_p"], dtype=np.float64)
            for i in range(N_CORES)]
    out = np.stack([outs[0] + outs[1] + outs[2] + outs[3],
                    outs[4] + outs[5] + outs[6] + outs[7]], axis=0)
    return out.reshape(B, C, H, W).astype(np.float32), res


def kernel(**inputs):
    out, _ = _run(trace=False, **inputs)
    return out


# revision 8
# speedup vs baseline: 1.1760x; 1.0358x over previous
"""Trainium2 Bass kernel for nn_Attention_54391465836966.

Math (per batch b):
  ctok = content_feat[b].raw_reshape(S,C) + pos         # [1024, 512]
  comp_tok[n] = components[n,b].raw_reshape(S,C) + pos
  q = ctok @ Wq ; k[n],v[n] = comp_tok[n] @ Wkv (split)
  per head h, comp n: P = exp(scale * q_h k_h^T); o_nh = (P @ v_nh) / rowsum(P)
  result = sum_n o_n ; s = (result + ctok) @ Wproj + bproj
  out = Wconv^T[C:] @ cf2d + Wconv^T[:C] @ s2d + bconv
    (s2d = raw reshape of the token-major s buffer; cf2d = content_feat[b])

Sharding: 8 cores <- (b, n) pairs; b = core//4, n = core%4.  Everything after
`result` is affine in the component partial, so each core applies the linear
tail to its own o_n (constant terms gated to the n==0 core via zeroed
per-core inputs) and the host sums the four partial outputs per batch.

Implementation notes:
- The reference's token matrices are RAW reshapes of [C,H,W] buffers (the
  same even/odd interleave as the final s->s2d reshape), so the host ships
  (raw_reshape(x) + pos).T directly (bf16); nothing on-chip ever transposes
  tokens.  Wconv is host-transposed as well.
- All matmul operands are bf16 (fp32 PSUM accumulate).  DMA volume is half
  of fp32, and transposes run at 1.0 cycles/row.
- Attention is ACT-bound (64 exps of [128,1024] ~= 1 us each), so the
  kT/qT matmuls of later head-pairs are interleaved INTO the attention
  stream to soak up spare PE cycles.  PSUM budget makes this possible:
  o is single-buffered and drained immediately with UNnormalized copies;
  normalization (recip batched per pair, one [128,S] multiply) happens in
  SBUF afterwards -> mm(2) + sc(2x2) + o(2) = 8 banks.
- Projection is computed transposed (stT = Wproj^T @ s_in^T) with head
  pairs packed into 128-partition tiles (full-K matmuls), bias via
  per-partition tensor_scalar on the PSUM drain.  The s->s2d raw-reshape
  is 32 PE transposes of stride-2 column slices, interleaved parity-major
  with the conv so output DMAs start early.  The cf half of the conv is
  emitted at the head of the tail to cover the last pair's normalization
  chain.
"""
import sys

sys.path.insert(0, "/opt/trn_rl_repo")

import numpy as np

N_CORES = 8
B, C, H, W = 2, 512, 32, 32
S = H * W  # 1024
NH, HD = 8, 64
SCALE = HD ** -0.5

_CACHE = {}


def _build():
    if "nc" in _CACHE:
        return _CACHE["nc"]
    from contextlib import ExitStack

    import concourse.bacc as bacc
    import concourse.mybir as mybir
    import concourse.tile as tile
    from concourse.masks import make_identity

    f32 = mybir.dt.float32
    bf16 = mybir.dt.bfloat16
    EXP = mybir.ActivationFunctionType.Exp
    MULT = mybir.AluOpType.mult
    ADD = mybir.AluOpType.add

    nc = bacc.Bacc("TRN2", target_bir_lowering=False, debug=False,
                   num_devices=N_CORES)

    din = lambda n, s, dt: nc.dram_tensor(n, s, dt, kind="ExternalInput").ap()
    cf = din("cf", [C, S], bf16)         # content_feat[b] raw [C,S] (conv only)
    ctokTd = din("ctokT", [C, S], bf16)  # (content_tok + pos).T, host-prepped
    compTd = din("compT", [C, S], bf16)  # (comp_tok + pos).T, host-prepped
    wq = din("wq", [C, C], bf16)
    wkv = din("wkv", [C, 2 * C], bf16)   # cols 0:C -> K, C:2C -> V
    wproj = din("wproj", [C, C], bf16)
    wconvT = din("wconvT", [2 * C, C], bf16)  # Wconv.T; rows C: zeroed n>0
    bprojT = din("bprojT", [C, 1], f32)       # zeroed n>0
    bconvT = din("bconvT", [C, 1], f32)       # zeroed n>0
    gate = din("gate", [128, 1], f32)         # 1.0 on n==0 cores else 0.0
    out_p = nc.dram_tensor("out_p", [C, S], f32, kind="ExternalOutput").ap()

    with tile.TileContext(nc) as tc, ExitStack() as ctx:
        main = ctx.enter_context(tc.tile_pool(name="main", bufs=1))

        # ---- constants ----
        ident32 = main.tile([128, 128], f32, tag="id32")
        make_identity(nc, ident32[:])
        ident = main.tile([128, 128], bf16, tag="ident")
        nc.vector.tensor_copy(ident[:], ident32[:])
        ones_bf = main.tile([128, 8], bf16, tag="ones")
        nc.gpsimd.memset(ones_bf[:], 1.0)
        g_sb = main.tile([128, 1], f32, tag="g")
        bpj_sb = [main.tile([128, 1], f32, tag=f"bpj{i}", name=f"bpj{i}")
                  for i in range(4)]
        bcv_sb = [main.tile([128, 1], f32, tag=f"bcv{i}", name=f"bcv{i}")
                  for i in range(4)]

        # ---- persistent SBUF tiles ----
        wkv_sb = [main.tile([128, 2 * C], bf16, tag=f"wkv{k}", name=f"wkv{k}")
                  for k in range(4)]
        wq_sb = [main.tile([128, C], bf16, tag=f"wq{k}", name=f"wq{k}")
                 for k in range(4)]
        comp_sb = [main.tile([128, S], bf16, tag=f"cm{k}", name=f"cm{k}")
                   for k in range(4)]
        cf_sb = [main.tile([128, S], bf16, tag=f"cf{k}", name=f"cf{k}")
                 for k in range(4)]
        ctokT = [main.tile([128, S], bf16, tag=f"ct{k}", name=f"ct{k}")
                 for k in range(4)]
        kT = [main.tile([128, S], bf16, tag=f"kt{j}", name=f"kt{j}")
              for j in range(4)]
        qT = [main.tile([128, S], bf16, tag=f"qt{j}", name=f"qt{j}")
              for j in range(4)]
        v_sb = [main.tile([128, 8 * 65], bf16, tag=f"v{t}", name=f"v{t}")
                for t in range(8)]
        wcc_sb = [main.tile([128, C], bf16, tag=f"wcc{k}", name=f"wcc{k}")
                  for k in range(4)]
        wcs_sb = [main.tile([128, C], bf16, tag=f"wcs{k}", name=f"wcs{k}")
                  for k in range(4)]
        wp_sb = [main.tile([128, C], bf16, tag=f"wp{k}", name=f"wp{k}")
                 for k in range(4)]
        outpart = [main.tile([128, S], f32, tag=f"op{oc}", name=f"op{oc}")
                   for oc in range(4)]
        # rtb[j]: normalized bf16 head pair (2j, 2j+1), later s_in^T chunk j
        rtb = [main.tile([128, S], bf16, tag=f"rt{j}", name=f"rt{j}")
               for j in range(4)]

        # ---- DMA emission order: attention-critical first ----
        for k in range(4):  # kT[0] inputs first (left halves + K cols)
            nc.sync.dma_start(comp_sb[k][:, 0:512],
                              compTd[128 * k:128 * (k + 1), 0:512])
            nc.sync.dma_start(wkv_sb[k][:, 0:C], wkv[128 * k:128 * (k + 1), 0:C])
        for k in range(4):
            nc.sync.dma_start(comp_sb[k][:, 512:S],
                              compTd[128 * k:128 * (k + 1), 512:S])
            nc.sync.dma_start(wkv_sb[k][:, C:2 * C],
                              wkv[128 * k:128 * (k + 1), C:2 * C])
        for k in range(4):
            nc.sync.dma_start(wq_sb[k][:], wq[128 * k:128 * (k + 1), :])
        for k in range(4):
            nc.sync.dma_start(ctokT[k][:], ctokTd[128 * k:128 * (k + 1), :])
        for k in range(4):
            nc.sync.dma_start(cf_sb[k][:], cf[128 * k:128 * (k + 1), :])
        for k in range(4):  # conv weights for the cf half (rows C:2C of Wconv.T)
            nc.sync.dma_start(wcc_sb[k][:],
                              wconvT[C + 128 * k:C + 128 * (k + 1), :])
        nc.sync.dma_start(g_sb[:], gate[:])
        for i in range(4):
            nc.sync.dma_start(bcv_sb[i][:], bconvT[128 * i:128 * (i + 1), :])
            nc.sync.dma_start(bpj_sb[i][:], bprojT[128 * i:128 * (i + 1), :])
        for k in range(4):  # tail weights last
            nc.sync.dma_start(wp_sb[k][:], wproj[128 * k:128 * (k + 1), :])
        for k in range(4):
            nc.sync.dma_start(wcs_sb[k][:], wconvT[128 * k:128 * (k + 1), :])

        # one PSUM pool spans startup + attention: mm(1x2) + sc(2x2) + o(2x1)
        # = 8 banks.  The mm tag stays live through attention so the later
        # head-pairs' kT/qT matmuls can interleave into ACT-bound stretches.
        with tc.tile_pool(name="psMain", bufs=1, space="PSUM") as ps:

            def emit_kq(j, dst, wsrc, act, cp_eng):
                for t in range(2):
                    acc = ps.tile([128, 512], f32, tag="mm", bufs=2)
                    for k in range(4):
                        nc.tensor.matmul(acc[:],
                                         wsrc[k][:, 128 * j:128 * (j + 1)],
                                         act[k][:, 512 * t:512 * (t + 1)],
                                         start=(k == 0), stop=(k == 3))
                    cp_eng(dst[j][:, 512 * t:512 * (t + 1)], acc[:])

            emit_kq(0, kT, wkv_sb, comp_sb, nc.vector.tensor_copy)
            emit_kq(0, qT, wq_sb, ctokT, nc.scalar.copy)

            # v (lhsT for the o matmuls; o(kt) consumes v[kt] in order)
            for t in range(8):
                nc.scalar.copy(
                    v_sb[t][:].rearrange("p (h e) -> p h e", h=8)[:, :, 64:65],
                    ones_bf[:, 0:8].rearrange("p (h o) -> p h o", o=1))
                acc = ps.tile([128, 512], f32, tag="mm", bufs=2)
                for k in range(4):
                    nc.tensor.matmul(acc[:], comp_sb[k][:, 128 * t:128 * (t + 1)],
                                     wkv_sb[k][:, C:2 * C],
                                     start=(k == 0), stop=(k == 3))
                nc.scalar.copy(
                    v_sb[t][:].rearrange("p (h e) -> p h e", h=8)[:, :, 0:64],
                    acc[:].rearrange("p (h d) -> p h d", h=8))

            # ---- attention ----
            # head order: pairs 0..3; within the LAST pair the odd head goes
            # first so the closing normalization chain is one op shorter.
            heads = [0, 1, 2, 3, 4, 5, 7, 6]
            sc_prev = None
            pair_state = {}
            for idx, h in enumerate(heads):
                jq, row = h // 2, 64 * (h % 2)
                o_ps = ps.tile([65, S], f32, tag="o", bufs=1)
                scs = []
                for kt in range(8):
                    if idx == 0 or kt > 0:
                        sc = ps.tile([128, S], f32, tag="sc", bufs=2)
                        for qc in range(2):
                            nc.tensor.matmul(
                                sc[:, 512 * qc:512 * (qc + 1)],
                                kT[jq][row:row + 64, 128 * kt:128 * (kt + 1)],
                                qT[jq][row:row + 64, 512 * qc:512 * (qc + 1)],
                                start=True, stop=True)
                    else:
                        sc = sc_prev
                    scs.append(sc)
                    # interleave next pair's kT/qT matmuls mid-head (after
                    # scores kt=3): even head slot carries kT, odd carries qT
                    if kt == 3 and idx < 6 and jq < 3:
                        if idx % 2 == 0:
                            emit_kq(jq + 1, kT, wkv_sb, comp_sb,
                                    nc.vector.tensor_copy)
                        else:
                            emit_kq(jq + 1, qT, wq_sb, ctokT, nc.scalar.copy)
                for kt in range(8):
                    if idx < NH - 1 and kt == 7:
                        # pre-emit next head's kt=0 scores (keeps ACT fed
                        # across the head boundary)
                        h2 = heads[idx + 1]
                        jq2, row2 = h2 // 2, 64 * (h2 % 2)
                        sc_prev = ps.tile([128, S], f32, tag="sc", bufs=2)
                        for qc in range(2):
                            nc.tensor.matmul(
                                sc_prev[:, 512 * qc:512 * (qc + 1)],
                                kT[jq2][row2:row2 + 64, 0:128],
                                qT[jq2][row2:row2 + 64,
                                        512 * qc:512 * (qc + 1)],
                                start=True, stop=True)
                    pt = main.tile([128, S], bf16, tag=f"pt{kt % 4}",
                                   name=f"pt{h}_{kt}")
                    nc.scalar.activation(pt[:], scs[kt][:], EXP, scale=SCALE)
                    for qc in range(2):
                        nc.tensor.matmul(
                            o_ps[:, 512 * qc:512 * (qc + 1)],
                            v_sb[kt][:, 65 * h:65 * h + 65],
                            pt[:, 512 * qc:512 * (qc + 1)],
                            start=(kt == 0), stop=(kt == 7))
                # drain o immediately (unnormalized) so the single o bank
                # frees; z row goes to the pair's z2 tile
                if jq not in pair_state:
                    rtf = main.tile([128, S], f32, tag="rtf", bufs=2,
                                    name=f"rtf{jq}")
                    zbc2 = main.tile([128, S], f32, tag="zb", bufs=2,
                                     name=f"zb{jq}")
                    pair_state[jq] = (rtf, zbc2)
                else:
                    rtf, zbc2 = pair_state[jq]
                nc.vector.tensor_copy(rtf[row:row + 64, :], o_ps[0:64, :])
                # per-head Z -> 1/Z -> broadcast into the pair-wide zbc2 half
                zE = main.tile([1, S], f32, tag="z", bufs=2, name=f"z{h}")
                zi = main.tile([1, S], f32, tag="zi", bufs=2, name=f"zi{h}")
                nc.vector.tensor_copy(zE[0:1, :], o_ps[64:65, :])
                nc.vector.reciprocal_approx_fast(zi[0:1, :], zE[0:1, :])
                if h % 2 == 0:
                    nc.gpsimd.partition_broadcast(zbc2[0:64, :], zi[0:1, :])
                else:
                    zscr = main.tile([64, S], f32, tag="zs", bufs=2,
                                     name=f"zs{jq}")
                    nc.gpsimd.partition_broadcast(zscr[0:64, :], zi[0:1, :])
                    nc.scalar.copy(zbc2[64:128, :], zscr[0:64, :])
                if idx % 2 == 1:  # pair complete -> normalize + s_in
                    nc.vector.tensor_mul(rtb[jq][:], rtf[:], zbc2[:])
                    # s_in^T[j] = rtb[j] + gate * ctokT[j]
                    nc.vector.scalar_tensor_tensor(
                        rtb[jq][:], ctokT[jq][:], g_sb[:, 0:1], rtb[jq][:],
                        MULT, ADD)

        # ---- tail ----
        # stT rides the dead wkv tags, s2d rides kT's
        stT = [main.tile([128, S], bf16, tag=f"wkv{cc}", name=f"stT{cc}")
               for cc in range(4)]
        s2d = [main.tile([128, S], bf16, tag=f"kt{jj}", name=f"s2d{jj}")
               for jj in range(4)]
        with tc.tile_pool(name="psTail", bufs=1, space="PSUM") as psT:
            # cf-half conv first: fills PE while the last pair normalizes
            for oc in range(4):
                for pc in range(2):
                    acc = psT.tile([128, 512], f32, tag="cva", bufs=2)
                    for k2 in range(4):
                        nc.tensor.matmul(acc[:],
                                         wcc_sb[k2][:, 128 * oc:128 * (oc + 1)],
                                         cf_sb[k2][:, 512 * pc:512 * (pc + 1)],
                                         start=(k2 == 0), stop=(k2 == 3))
                    nc.vector.tensor_scalar_add(
                        outpart[oc][:, 512 * pc:512 * (pc + 1)], acc[:],
                        bcv_sb[oc][:, 0:1])

            # stT[cc] = Wproj^T @ s_in^T (+ bproj per partition)
            def emit_stT(cc):
                for half in range(2):
                    acc = psT.tile([128, 512], f32, tag="st", bufs=2)
                    for j in range(4):
                        nc.tensor.matmul(
                            acc[:],
                            wp_sb[j][:, 128 * cc:128 * (cc + 1)],
                            rtb[j][:, 512 * half:512 * (half + 1)],
                            start=(j == 0), stop=(j == 3))
                    nc.vector.tensor_scalar_add(
                        stT[cc][:, 512 * half:512 * (half + 1)], acc[:],
                        bpj_sb[cc][:, 0:1])

            # s2d repack: s2d[i, c + 512*par] = stT[c, 2i + par]; parity-major
            # so the pc=0 conv (and its output DMAs) can start early
            cp_engs = [nc.scalar.copy, nc.vector.tensor_copy]
            ncp = [0]

            def emit_T(cc, par):
                ev = stT[cc][:].rearrange("p (t two) -> p two t", two=2)
                for jj in range(4):
                    tp = psT.tile([128, 128], bf16, tag="tp", bufs=4)
                    nc.tensor.transpose(
                        tp[:], ev[:, par, 128 * jj:128 * (jj + 1)], ident[:])
                    cp_engs[ncp[0] % 2](
                        s2d[jj][:, 512 * par + 128 * cc:
                                512 * par + 128 * (cc + 1)], tp[:])
                    ncp[0] += 1

            def emit_conv_s(pc):
                for oc in range(4):
                    acc = psT.tile([128, 512], f32, tag="cva", bufs=2)
                    for jj in range(4):
                        nc.tensor.matmul(acc[:],
                                         wcs_sb[jj][:, 128 * oc:128 * (oc + 1)],
                                         s2d[jj][:, 512 * pc:512 * (pc + 1)],
                                         start=(jj == 0), stop=(jj == 3))
                    nc.vector.tensor_add(
                        outpart[oc][:, 512 * pc:512 * (pc + 1)],
                        outpart[oc][:, 512 * pc:512 * (pc + 1)], acc[:])
                    nc.sync.dma_start(
                        out_p[128 * oc:128 * (oc + 1),
                              512 * pc:512 * (pc + 1)],
                        outpart[oc][:, 512 * pc:512 * (pc + 1)])

            emit_stT(0)
            emit_stT(1)
            emit_T(0, 0)
            emit_stT(2)
            emit_T(1, 0)
            emit_stT(3)
            emit_T(2, 0)
            emit_T(3, 0)
            emit_conv_s(0)
            for cc in range(4):
                emit_T(cc, 1)
            emit_conv_s(1)

    nc.compile()
    _CACHE["nc"] = nc
    return nc


def _shard_inputs(content_feat, components, pos_emb, Wq, Wkv, Wproj, bproj,
                  Wconv, bconv):
    import ml_dtypes

    bf = ml_dtypes.bfloat16
    f = np.float32
    pos2 = np.asarray(pos_emb, dtype=f).reshape(S, C)
    wq2 = np.asarray(Wq, dtype=f).astype(bf)
    wkv2 = np.asarray(Wkv, dtype=f).astype(bf)
    wp2 = np.asarray(Wproj, dtype=f).astype(bf)
    wcT = np.ascontiguousarray(np.asarray(Wconv, dtype=f).T).astype(bf)
    wcT_rest = wcT.copy()
    wcT_rest[C:, :] = 0
    bpj = np.ascontiguousarray(np.asarray(bproj, dtype=f).reshape(C, 1))
    bcv = np.ascontiguousarray(np.asarray(bconv, dtype=f).reshape(C, 1))
    zeros = np.zeros((C, 1), dtype=f)
    in_maps = []
    for core in range(N_CORES):
        b, n = core // 4, core % 4
        first = n == 0
        in_maps.append({
            "cf": np.ascontiguousarray(
                np.asarray(content_feat[b], dtype=f).reshape(C, S)).astype(bf),
            "ctokT": np.ascontiguousarray(
                (np.asarray(content_feat[b], dtype=f).reshape(S, C)
                 + pos2).T).astype(bf),
            "compT": np.ascontiguousarray(
                (np.asarray(components[n, b], dtype=f).reshape(S, C)
                 + pos2).T).astype(bf),
            "wq": wq2,
            "wkv": wkv2,
            "wproj": wp2,
            "wconvT": wcT if first else wcT_rest,
            "bprojT": bpj if first else zeros,
            "bconvT": bcv if first else zeros,
            "gate": np.full((128, 1), 1.0 if first else 0.0, dtype=f),
        })
    return in_maps


def _run(trace=False, **inputs):
    from concourse.bass_utils import run_bass_kernel_spmd

    nc = _build()
    in_maps = _shard_inputs(**inputs)
    res = run_bass_kernel_spmd(nc, in_maps, list(range(N_CORES)), trace=trace)
    outs = [np.asarray(res.results[i]["out_p"], dtype=np.float64)
            for i in range(N_CORES)]
    out = np.stack([outs[0] + outs[1] + outs[2] + outs[3],
                    outs[4] + outs[5] + outs[6] + outs[7]], axis=0)
    return out.reshape(B, C, H, W).astype(np.float32), res


def kernel(**inputs):
    out, _ = _run(trace=False, **inputs)
    return out


# revision 12
# speedup vs baseline: 1.2954x; 1.1015x over previous
"""Trainium2 Bass kernel for nn_Attention_54391465836966.

Math (per batch b):
  ctok = content_feat[b].raw_reshape(S,C) + pos         # [1024, 512]
  comp_tok[n] = components[n,b].raw_reshape(S,C) + pos
  q = ctok @ Wq ; k[n],v[n] = comp_tok[n] @ Wkv (split)
  per head h, comp n: P = exp(scale * q_h k_h^T); o_nh = (P @ v_nh) / rowsum(P)
  result = sum_n o_n ; s = (result + ctok) @ Wproj + bproj
  out = Wconv^T[C:] @ cf2d + Wconv^T[:C] @ s2d + bconv
    (s2d = raw reshape of the token-major s buffer; cf2d = content_feat[b])

Sharding: 8 cores <- (b, n) pairs; b = core//4, n = core%4.  Everything after
`result` is affine in the component partial, so each core applies the linear
tail to its own o_n (constant terms gated to the n==0 core via zeroed
per-core inputs) and the host sums the four partial outputs per batch.

Implementation notes:
- The reference's token matrices are RAW reshapes of [C,H,W] buffers (the
  same even/odd interleave as the final s->s2d reshape), so the host ships
  (raw_reshape(x) + pos).T directly (bf16); nothing on-chip ever transposes
  tokens.  Wconv is host-transposed as well.
- All matmul operands are bf16 (fp32 PSUM accumulate).  DMA volume is half
  of fp32, and transposes run at 1.0 cycles/row.
- Attention is ACT-bound (64 exps of [128,1024] ~= 1 us each), so the
  kT/qT matmuls of later head-pairs are interleaved INTO the attention
  stream to soak up spare PE cycles.  PSUM budget makes this possible:
  o is single-buffered and drained immediately with UNnormalized copies;
  normalization (recip batched per pair, one [128,S] multiply) happens in
  SBUF afterwards -> mm(2) + sc(2x2) + o(2) = 8 banks.
- Projection is computed transposed (stT = Wproj^T @ s_in^T) with head
  pairs packed into 128-partition tiles (full-K matmuls), bias via
  per-partition tensor_scalar on the PSUM drain.  The s->s2d raw-reshape
  is 32 PE transposes of stride-2 column slices, interleaved parity-major
  with the conv so output DMAs start early.  The cf half of the conv is
  emitted at the head of the tail to cover the last pair's normalization
  chain.
"""
import sys

sys.path.insert(0, "/opt/trn_rl_repo")

import numpy as np

N_CORES = 8
B, C, H, W = 2, 512, 32, 32
S = H * W  # 1024
NH, HD = 8, 64
SCALE = HD ** -0.5

_CACHE = {}


def _build():
    if "nc" in _CACHE:
        return _CACHE["nc"]
    from contextlib import ExitStack

    import concourse.bacc as bacc
    import concourse.mybir as mybir
    import concourse.tile as tile
    from concourse.masks import make_identity

    f32 = mybir.dt.float32
    bf16 = mybir.dt.bfloat16
    EXP = mybir.ActivationFunctionType.Exp
    MULT = mybir.AluOpType.mult
    ADD = mybir.AluOpType.add

    nc = bacc.Bacc("TRN2", target_bir_lowering=False, debug=False,
                   num_devices=N_CORES)

    din = lambda n, s, dt: nc.dram_tensor(n, s, dt, kind="ExternalInput").ap()
    cf = din("cf", [C, S], bf16)         # content_feat[b] raw [C,S] (conv only)
    ctokTd = din("ctokT", [C, S], bf16)  # (content_tok + pos).T, host-prepped
    compTd = din("compT", [C, S], bf16)  # (comp_tok + pos).T, host-prepped
    wq = din("wq", [C, C], bf16)
    wkv = din("wkv", [C, 2 * C], bf16)   # cols 0:C -> K, C:2C -> V
    wproj = din("wproj", [C, C], bf16)
    wconvT = din("wconvT", [2 * C, C], bf16)  # Wconv.T; rows C: zeroed n>0
    bprojT = din("bprojT", [C, 1], f32)       # zeroed n>0
    bconvT = din("bconvT", [C, 1], f32)       # zeroed n>0
    gate = din("gate", [128, 1], f32)         # 1.0 on n==0 cores else 0.0
    out_p = nc.dram_tensor("out_p", [C, S], f32, kind="ExternalOutput").ap()

    with tile.TileContext(nc) as tc, ExitStack() as ctx:
        main = ctx.enter_context(tc.tile_pool(name="main", bufs=1))

        # ---- constants ----
        ident32 = main.tile([128, 128], f32, tag="id32")
        make_identity(nc, ident32[:])
        ident = main.tile([128, 128], bf16, tag="ident")
        nc.vector.tensor_copy(ident[:], ident32[:])
        ones_bf = main.tile([128, 8], bf16, tag="ones")
        nc.gpsimd.memset(ones_bf[:], 1.0)
        g_sb = main.tile([128, 1], f32, tag="g")

        # ---- persistent SBUF tiles (one wide tile per DRAM tensor so each
        # input is a SINGLE DMA: per-DMA cost is ~650ns serial SP.SEQ +
        # ~625ns HWDGE + 900ns sem latency, so fewer/bigger wins) ----
        wkv_all = main.tile([128, 4 * 2 * C], bf16, tag="wkva", name="wkva")
        wq_all = main.tile([128, 4 * C], bf16, tag="wqa", name="wqa")
        comp_all = main.tile([128, 4 * S], bf16, tag="cma", name="cma")
        cf_all = main.tile([128, 4 * S], bf16, tag="cfa", name="cfa")
        ctok_all = main.tile([128, 4 * S], bf16, tag="cta", name="cta")
        wc_all = main.tile([128, 8 * C], bf16, tag="wca", name="wca")
        wp_all = main.tile([128, 4 * C], bf16, tag="wpa", name="wpa")
        bpj_all = main.tile([128, 4], f32, tag="bpja", name="bpja")
        bcv_all = main.tile([128, 4], f32, tag="bcva", name="bcva")
        # chunk views (k-th 128-row block of the [C,*] DRAM tensor)
        wkv_sb = [wkv_all[:, 2 * C * k:2 * C * (k + 1)] for k in range(4)]
        wq_sb = [wq_all[:, C * k:C * (k + 1)] for k in range(4)]
        comp_sb = [comp_all[:, S * k:S * (k + 1)] for k in range(4)]
        cf_sb = [cf_all[:, S * k:S * (k + 1)] for k in range(4)]
        ctokT = [ctok_all[:, S * k:S * (k + 1)] for k in range(4)]
        wcs_sb = [wc_all[:, C * k:C * (k + 1)] for k in range(4)]
        wcc_sb = [wc_all[:, C * (4 + k):C * (5 + k)] for k in range(4)]
        wp_sb = [wp_all[:, C * k:C * (k + 1)] for k in range(4)]
        bpj_sb = [bpj_all[:, i:i + 1] for i in range(4)]
        bcv_sb = [bcv_all[:, i:i + 1] for i in range(4)]
        kT = [main.tile([128, S], bf16, tag=f"kt{j}", name=f"kt{j}")
              for j in range(4)]
        qT = [main.tile([128, S], bf16, tag=f"qt{j}", name=f"qt{j}")
              for j in range(4)]
        v_sb = [main.tile([128, 8 * 65], bf16, tag=f"v{t}", name=f"v{t}")
                for t in range(8)]
        outpart = [main.tile([128, S], f32, tag=f"op{oc}", name=f"op{oc}")
                   for oc in range(4)]
        # rtb[j]: normalized bf16 head pair (2j, 2j+1), later s_in^T chunk j
        rtb = [main.tile([128, S], bf16, tag=f"rt{j}", name=f"rt{j}")
               for j in range(4)]

        # ---- DMA emission order: attention-critical first ----
        def dma_merged(dst_tile, src_ap, k):
            src3 = src_ap.rearrange("(k p) s -> p k s", k=k)
            dst3 = dst_tile[:].rearrange("p (k s) -> p k s", k=k)
            nc.sync.dma_start(dst3[:, :, :], src3[:, :, :])

        dma_merged(comp_all, compTd, 4)
        dma_merged(wkv_all, wkv, 4)
        dma_merged(wq_all, wq, 4)
        dma_merged(ctok_all, ctokTd, 4)
        dma_merged(cf_all, cf, 4)
        dma_merged(wc_all, wconvT, 8)
        nc.sync.dma_start(g_sb[:], gate[:])
        dma_merged(bcv_all, bconvT, 4)
        dma_merged(bpj_all, bprojT, 4)
        dma_merged(wp_all, wproj, 4)

        # one PSUM pool spans startup + attention: mm(1x2) + sc(2x2) + o(2x1)
        # = 8 banks.  The mm tag stays live through attention so the later
        # head-pairs' kT/qT matmuls can interleave into ACT-bound stretches.
        with tc.tile_pool(name="psMain", bufs=1, space="PSUM") as ps:

            def emit_kq(j, dst, wsrc, act, cp_eng):
                for t in range(2):
                    acc = ps.tile([128, 512], f32, tag="mm", bufs=2)
                    for k in range(4):
                        nc.tensor.matmul(acc[:],
                                         wsrc[k][:, 128 * j:128 * (j + 1)],
                                         act[k][:, 512 * t:512 * (t + 1)],
                                         start=(k == 0), stop=(k == 3))
                    cp_eng(dst[j][:, 512 * t:512 * (t + 1)], acc[:])

            emit_kq(0, kT, wkv_sb, comp_sb, nc.vector.tensor_copy)
            emit_kq(0, qT, wq_sb, ctokT, nc.scalar.copy)

            # v (lhsT for the o matmuls; o(kt) consumes v[kt] in order)
            for t in range(8):
                nc.scalar.copy(
                    v_sb[t][:].rearrange("p (h e) -> p h e", h=8)[:, :, 64:65],
                    ones_bf[:, 0:8].rearrange("p (h o) -> p h o", o=1))
                acc = ps.tile([128, 512], f32, tag="mm", bufs=2)
                for k in range(4):
                    nc.tensor.matmul(acc[:], comp_sb[k][:, 128 * t:128 * (t + 1)],
                                     wkv_sb[k][:, C:2 * C],
                                     start=(k == 0), stop=(k == 3))
                nc.scalar.copy(
                    v_sb[t][:].rearrange("p (h e) -> p h e", h=8)[:, :, 0:64],
                    acc[:].rearrange("p (h d) -> p h d", h=8))

            # ---- attention ----
            # head order: pairs 0..3; within the LAST pair the odd head goes
            # first so the closing normalization chain is one op shorter.
            heads = [0, 1, 2, 3, 4, 5, 7, 6]
            sc_prev = None
            pair_state = {}
            for idx, h in enumerate(heads):
                jq, row = h // 2, 64 * (h % 2)
                o_ps = ps.tile([65, S], f32, tag="o", bufs=1)
                scs = []
                for kt in range(8):
                    if idx == 0 or kt > 0:
                        sc = ps.tile([128, S], f32, tag="sc", bufs=2)
                        for qc in range(2):
                            nc.tensor.matmul(
                                sc[:, 512 * qc:512 * (qc + 1)],
                                kT[jq][row:row + 64, 128 * kt:128 * (kt + 1)],
                                qT[jq][row:row + 64, 512 * qc:512 * (qc + 1)],
                                start=True, stop=True)
                    else:
                        sc = sc_prev
                    scs.append(sc)
                    # interleave next pair's kT/qT matmuls mid-head (after
                    # scores kt=3): even head slot carries kT, odd carries qT
                    if kt == 3 and idx < 6 and jq < 3:
                        if idx % 2 == 0:
                            emit_kq(jq + 1, kT, wkv_sb, comp_sb,
                                    nc.vector.tensor_copy)
                        else:
                            emit_kq(jq + 1, qT, wq_sb, ctokT,
                                    nc.vector.tensor_copy)
                for kt in range(8):
                    if idx < NH - 1 and kt == 7:
                        # pre-emit next head's kt=0 scores (keeps ACT fed
                        # across the head boundary)
                        h2 = heads[idx + 1]
                        jq2, row2 = h2 // 2, 64 * (h2 % 2)
                        sc_prev = ps.tile([128, S], f32, tag="sc", bufs=2)
                        for qc in range(2):
                            nc.tensor.matmul(
                                sc_prev[:, 512 * qc:512 * (qc + 1)],
                                kT[jq2][row2:row2 + 64, 0:128],
                                qT[jq2][row2:row2 + 64,
                                        512 * qc:512 * (qc + 1)],
                                start=True, stop=True)
                    pt = main.tile([128, S], bf16, tag=f"pt{kt % 4}",
                                   name=f"pt{h}_{kt}")
                    nc.scalar.activation(pt[:], scs[kt][:], EXP, scale=SCALE)
                    for qc in range(2):
                        nc.tensor.matmul(
                            o_ps[:, 512 * qc:512 * (qc + 1)],
                            v_sb[kt][:, 65 * h:65 * h + 65],
                            pt[:, 512 * qc:512 * (qc + 1)],
                            start=(kt == 0), stop=(kt == 7))
                # drain o immediately (unnormalized) so the single o bank
                # frees; z row goes to the pair's z2 tile
                if jq not in pair_state:
                    rtf = main.tile([128, S], f32, tag="rtf", bufs=2,
                                    name=f"rtf{jq}")
                    zbc2 = main.tile([128, S], f32, tag="zb", bufs=2,
                                     name=f"zb{jq}")
                    pair_state[jq] = (rtf, zbc2)
                else:
                    rtf, zbc2 = pair_state[jq]
                nc.vector.tensor_copy(rtf[row:row + 64, :], o_ps[0:64, :])
                # per-head Z -> 1/Z -> broadcast into the pair-wide zbc2 half
                zE = main.tile([1, S], f32, tag="z", bufs=2, name=f"z{h}")
                zi = main.tile([1, S], f32, tag="zi", bufs=2, name=f"zi{h}")
                nc.vector.tensor_copy(zE[0:1, :], o_ps[64:65, :])
                nc.vector.reciprocal_approx_fast(zi[0:1, :], zE[0:1, :])
                if h % 2 == 0:
                    nc.gpsimd.partition_broadcast(zbc2[0:64, :], zi[0:1, :])
                else:
                    zscr = main.tile([64, S], f32, tag="zs", bufs=2,
                                     name=f"zs{jq}")
                    nc.gpsimd.partition_broadcast(zscr[0:64, :], zi[0:1, :])
                    nc.vector.tensor_copy(zbc2[64:128, :], zscr[0:64, :])
                if idx % 2 == 1:  # pair complete -> normalize + s_in
                    nc.vector.tensor_mul(rtb[jq][:], rtf[:], zbc2[:])
                    # s_in^T[j] = rtb[j] + gate * ctokT[j]
                    nc.vector.scalar_tensor_tensor(
                        rtb[jq][:], ctokT[jq][:], g_sb[:, 0:1], rtb[jq][:],
                        MULT, ADD)

            # cf-half conv inside the attention pool (mm tag): fills PE while
            # the last pair's normalization chain completes, with no pool-
            # transition barrier
            for oc in range(4):
                for pc in range(2):
                    acc = ps.tile([128, 512], f32, tag="mm", bufs=2)
                    for k2 in range(4):
                        nc.tensor.matmul(acc[:],
                                         wcc_sb[k2][:, 128 * oc:128 * (oc + 1)],
                                         cf_sb[k2][:, 512 * pc:512 * (pc + 1)],
                                         start=(k2 == 0), stop=(k2 == 3))
                    nc.vector.tensor_scalar_add(
                        outpart[oc][:, 512 * pc:512 * (pc + 1)], acc[:],
                        bcv_sb[oc][:, 0:1])

        # ---- tail ----
        # stT rides the dead wkv mega-tag, s2d rides kT's
        stT_all = main.tile([128, 4 * S], bf16, tag="wkva", name="stT_all")
        stT = [stT_all[:, S * cc:S * (cc + 1)] for cc in range(4)]
        s2d = [main.tile([128, S], bf16, tag=f"kt{jj}", name=f"s2d{jj}")
               for jj in range(4)]
        with tc.tile_pool(name="psTail", bufs=1, space="PSUM") as psT:
            # stT[cc] = Wproj^T @ s_in^T (+ bproj per partition)
            def emit_stT(cc):
                for half in range(2):
                    acc = psT.tile([128, 512], f32, tag="st", bufs=2)
                    for j in range(4):
                        nc.tensor.matmul(
                            acc[:],
                            wp_sb[j][:, 128 * cc:128 * (cc + 1)],
                            rtb[j][:, 512 * half:512 * (half + 1)],
                            start=(j == 0), stop=(j == 3))
                    nc.vector.tensor_scalar_add(
                        stT[cc][:, 512 * half:512 * (half + 1)], acc[:],
                        bpj_sb[cc][:, 0:1])

            # s2d repack: s2d[i, c + 512*par] = stT[c, 2i + par]; parity-major
            # so the pc=0 conv (and its output DMAs) can start early
            def emit_T(cc, par):
                ev = stT[cc].rearrange("p (t two) -> p two t", two=2)
                for jj in range(4):
                    tp = psT.tile([128, 128], bf16, tag="tp", bufs=4)
                    nc.tensor.transpose(
                        tp[:], ev[:, par, 128 * jj:128 * (jj + 1)], ident[:])
                    nc.scalar.copy(
                        s2d[jj][:, 512 * par + 128 * cc:
                                512 * par + 128 * (cc + 1)], tp[:])

            def emit_conv_s(pc):
                for oc in range(4):
                    acc = psT.tile([128, 512], f32, tag="cva", bufs=2)
                    for jj in range(4):
                        nc.tensor.matmul(acc[:],
                                         wcs_sb[jj][:, 128 * oc:128 * (oc + 1)],
                                         s2d[jj][:, 512 * pc:512 * (pc + 1)],
                                         start=(jj == 0), stop=(jj == 3))
                    nc.vector.tensor_add(
                        outpart[oc][:, 512 * pc:512 * (pc + 1)],
                        outpart[oc][:, 512 * pc:512 * (pc + 1)], acc[:])
                    nc.sync.dma_start(
                        out_p[128 * oc:128 * (oc + 1),
                              512 * pc:512 * (pc + 1)],
                        outpart[oc][:, 512 * pc:512 * (pc + 1)])

            emit_stT(0)
            emit_stT(1)
            emit_T(0, 0)
            emit_stT(2)
            emit_T(1, 0)
            emit_stT(3)
            emit_T(2, 0)
            emit_T(3, 0)
            emit_conv_s(0)
            for cc in range(4):
                emit_T(cc, 1)
            emit_conv_s(1)

    nc.compile()
    _CACHE["nc"] = nc
    return nc


def _shard_inputs(content_feat, components, pos_emb, Wq, Wkv, Wproj, bproj,
                  Wconv, bconv):
    import ml_dtypes

    bf = ml_dtypes.bfloat16
    f = np.float32
    pos2 = np.asarray(pos_emb, dtype=f).reshape(S, C)
    wq2 = np.asarray(Wq, dtype=f).astype(bf)
    wkv2 = np.asarray(Wkv, dtype=f).astype(bf)
    wp2 = np.asarray(Wproj, dtype=f).astype(bf)
    wcT = np.ascontiguousarray(np.asarray(Wconv, dtype=f).T).astype(bf)
    wcT_rest = wcT.copy()
    wcT_rest[C:, :] = 0
    bpj = np.ascontiguousarray(np.asarray(bproj, dtype=f).reshape(C, 1))
    bcv = np.ascontiguousarray(np.asarray(bconv, dtype=f).reshape(C, 1))
    zeros = np.zeros((C, 1), dtype=f)
    in_maps = []
    for core in range(N_CORES):
        b, n = core // 4, core % 4
        first = n == 0
        in_maps.append({
            "cf": np.ascontiguousarray(
                np.asarray(content_feat[b], dtype=f).reshape(C, S)).astype(bf),
            "ctokT": np.ascontiguousarray(
                (np.asarray(content_feat[b], dtype=f).reshape(S, C)
                 + pos2).T).astype(bf),
            "compT": np.ascontiguousarray(
                (np.asarray(components[n, b], dtype=f).reshape(S, C)
                 + pos2).T).astype(bf),
            "wq": wq2,
            "wkv": wkv2,
            "wproj": wp2,
            "wconvT": wcT if first else wcT_rest,
            "bprojT": bpj if first else zeros,
            "bconvT": bcv if first else zeros,
            "gate": np.full((128, 1), 1.0 if first else 0.0, dtype=f),
        })
    return in_maps


def _run(trace=False, **inputs):
    from concourse.bass_utils import run_bass_kernel_spmd

    nc = _build()
    in_maps = _shard_inputs(**inputs)
    res = run_bass_kernel_spmd(nc, in_maps, list(range(N_CORES)), trace=trace)
    outs = [np.asarray(res.results[i]["out_p"], dtype=np.float64)
            for i in range(N_CORES)]
    out = np.stack([outs[0] + outs[1] + outs[2] + outs[3],
                    outs[4] + outs[5] + outs[6] + outs[7]], axis=0)
    return out.reshape(B, C, H, W).astype(np.float32), res


def kernel(**inputs):
    out, _ = _run(trace=False, **inputs)
    return out
